# revision 22
# baseline (speedup 1.0000x reference)
"""Multi-head attention (RoPE + causal softmax) Trainium2 Bass kernel.

Sharding: 8 cores = 4 batches x 2 head-groups (tensor-parallel over heads).
Each core computes, for its (batch b, head-group g):
    Q/K/V projections for its 8 heads, RoPE, causal attention, and a
    partial output projection with its 512-row slice of W_O^T.
Host sums the two partial outputs per batch.

v13 (318us, from the 509us v2 baseline):
  - Attention inner loop software-pipelined: scores(kb+1) is emitted
    before PV(kb), so the ACT exp of step kb runs while the PE computes
    the next step's scores (the v2 loop serialized PE->ACT->PE per step).
  - fp8e4 DoubleRow (0.5 cyc/row) for the QKV projections of passes 1-3
    (x and 32x-rescaled W in fp8; 4 dual-k-tile matmuls per chain) and
    for off-diagonal PV key-block pairs (V padded to 80 cols for the
    dual-fp8 16B ldweights stride rule; exp writes fp8 P directly).
    Pass 0 and tile-0 attention stay fp16 so early tokens (small
    softmax fan-in, no error averaging) keep fp16-grade values; the
    diagonal blocks of tiles 1-3 also run as fp8 pairs (fp8 mask-mul
    on the exp output, u8-bitcast zeroing of below-diagonal strips).
    Scores stay fp16: dual-fp8 with 32-partition k-tiles measured
    ~600ns vs the model's 213ns, slower than the fp16 64-contraction
    form (each scores matmul is ~512cyc@1.2GHz + ~170ns fixed cost,
    capped by the 2KB PSUM bank; the PE never reaches its 2.4GHz
    p-state in this dependency-paced loop).
  - Scheduling: only proj pass 0 runs upfront; pass t+1 (early), then
    normalize(t-1), then outproj(t-1) spread as fine-grained fillers
    inside tile t; W_O load deferred into tile 0.
  - Normalization: per-tile denominators banked at 32-aligned rows,
    reciprocal as exp(-ln(x)) on the ACT engine (the 3.3us DVE
    reciprocal at tile boundaries was head-of-line-blocking the rope
    tails, which held PSUM slots and stalled the in-order PE queue),
    then one half-indicator [65,128] broadcast matmul + one in-place
    mul per chunk (rec pool slots keep rows 1-63 zeroed across
    generations so the contraction stays junk-free); at tiles fp16
    with a /16 guard scale against overflow.
  - Startup: pass-0 W/x DMAs issue k-interleaved so the first
    projection matmul starts after ~2 blocks, not the full 8MB.
"""

import os
import sys
import types

import ml_dtypes
import numpy as np

D_MODEL = 1024
NUM_HEADS = 16
HEAD_DIM = 64
THETA = 10000.0
BATCH = 4
SEQ = 2048
N_CORES = 8
HPC = 8          # heads per core
NCHUNK = HPC // 2  # 128-row chunks of the per-core 512 head dims
NQT = SEQ // 512   # 512-wide seq tiles
NSC = SEQ // 128   # 128-row seq chunks
KD = D_MODEL // 128  # contraction chunks for projections


# ---------------------------------------------------------------------------
# environment shims (axon container: missing antenv.axon_hooks; walrus here
# supports only 1 sync-wait per instruction)
# ---------------------------------------------------------------------------
def _install_axon_hooks():
    import antenv

    if hasattr(antenv, "axon_hooks"):
        return
    mod = types.ModuleType("antenv.axon_hooks")
    _hook = [None]
    mod.set_axon_ntff_profile_hook = lambda h: _hook.__setitem__(0, h)
    mod.get_axon_ntff_profile_hook = lambda: _hook[0]
    sys.modules["antenv.axon_hooks"] = mod
    antenv.axon_hooks = mod
    try:
        from trn_agent_boot.trn_boot import _ntff_profile_via_ctypes

        mod.set_axon_ntff_profile_hook(
            _ntff_profile_via_ctypes("/opt/axon/libaxon_pjrt.so")
        )
    except Exception:
        pass


def _install_drain_patch():
    import concourse.mybir as mybir
    import concourse.tile as tilemod

    if getattr(tilemod.TileContext, "_drain_patch_installed", False):
        return

    def _drain_and_barrier(self, tick_clock, wait_clock):
        carrier = self.nc.sync.nop(nofuse=True)
        wait_clock.add_sem_waits(
            carrier.ins, tilemod.ScopedClock({None: tick_clock.global_clock})
        )
        si = carrier.ins.sync_info
        if si is not None and si.on_wait and len(si.on_wait) > 1:
            waits = list(si.on_wait)
            carrier.ins.sync_info = mybir.SyncInfo(
                on_wait=[waits[0]], on_update=list(si.on_update or [])
            )
            for w in waits[1:]:
                nop = self.nc.sync.nop(nofuse=True)
                nop.ins.sync_info = mybir.SyncInfo(on_wait=[w], on_update=[])
        self.nc.sync.drain()

        self.nc.all_engine_barrier()
        assert self.sems is not None
        popped = self.nc._tile_sem_poison_stack.pop()
        assert popped is self._sem_poison
        self.nc.clear_and_free_semaphores(list(self.sems.allocated().values()))
        self.nc.all_engine_barrier()

    tilemod.TileContext._drain_and_barrier = _drain_and_barrier
    tilemod.TileContext._drain_patch_installed = True


def _split_sync_waits(nc, max_waits=1):
    """Hoist excess per-instruction sem waits onto same-engine NoOps."""
    import concourse.mybir as mybir

    n_added = 0
    for fn in nc.m.functions:
        for bb in fn.blocks:
            insts = bb.instructions
            new_list = []
            changed = False
            for inst in insts:
                si = inst.sync_info
                waits = list(si.on_wait) if si is not None and si.on_wait else []
                if (
                    len(waits) > max_waits
                    and inst.engine != mybir.EngineType.Unassigned
                ):
                    keep = waits[-max_waits:]
                    extra = waits[:-max_waits]
                    while extra:
                        chunk, extra = extra[:max_waits], extra[max_waits:]
                        nop = mybir.InstNoOp(
                            name=f"I-waitsplit-{n_added}", ins=[], outs=[]
                        )
                        nop.engine = inst.engine
                        nop.bass_nofuse = True
                        nop.sync_info = mybir.SyncInfo(on_wait=chunk, on_update=[])
                        new_list.append(nop)
                        n_added += 1
                    inst.sync_info = mybir.SyncInfo(
                        on_wait=keep, on_update=list(si.on_update or [])
                    )
                    changed = True
                new_list.append(inst)
            if changed:
                bb.instructions = new_list
    return n_added


# ---------------------------------------------------------------------------
# device program
# ---------------------------------------------------------------------------
def _build_program():
    import concourse.bass as bass
    import concourse.mybir as mybir
    import concourse.tile as tile

    f32 = mybir.dt.float32
    f32r = mybir.dt.float32r
    f16 = mybir.dt.float16
    f8 = mybir.dt.float8e4
    u8 = mybir.dt.uint8
    DR = mybir.MatmulPerfMode.DoubleRow
    Exp = mybir.ActivationFunctionType.Exp
    Ln = mybir.ActivationFunctionType.Ln
    EXP_SCALE = 2.0 ** -13  # folds the 1/sqrt(64) score scale and the
    # 32x fp8-range rescale of W_Q/W_K (32*32*8 = 8192)

    nc = bass.Bass("TRN2", target_bir_lowering=False, debug=False,
                   num_devices=N_CORES)

    xT = nc.dram_tensor("xT", [D_MODEL, SEQ], f8, kind="ExternalInput").ap()
    wqT = nc.dram_tensor("wqT", [D_MODEL, 512], f8, kind="ExternalInput").ap()
    wkT = nc.dram_tensor("wkT", [D_MODEL, 512], f8, kind="ExternalInput").ap()
    wvT = nc.dram_tensor("wvT", [D_MODEL, 512], f8, kind="ExternalInput").ap()
    wq16T = nc.dram_tensor("wq16T", [D_MODEL, 512], f16,
                           kind="ExternalInput").ap()
    wk16T = nc.dram_tensor("wk16T", [D_MODEL, 512], f16,
                           kind="ExternalInput").ap()
    wv16T = nc.dram_tensor("wv16T", [D_MODEL, 512], f16,
                           kind="ExternalInput").ap()
    x16T = nc.dram_tensor("x16T", [D_MODEL, 512], f16,
                          kind="ExternalInput").ap()
    woT = nc.dram_tensor("woT", [512, D_MODEL], f16, kind="ExternalInput").ap()
    cos_d = nc.dram_tensor("cos_t", [128, SEQ], f16, kind="ExternalInput").ap()
    sin_d = nc.dram_tensor("sin_t", [128, SEQ], f16, kind="ExternalInput").ap()
    mask_d = nc.dram_tensor("bigmask", [128, 128], f16, kind="ExternalInput").ap()
    mask8_d = nc.dram_tensor("bigmask8", [128, 128], f8,
                             kind="ExternalInput").ap()
    ones_d = nc.dram_tensor("ones64", [1, 64], f32r, kind="ExternalInput").ap()
    halves_d = nc.dram_tensor("halves65", [65, 128], f32r,
                              kind="ExternalInput").ap()
    out_d = nc.dram_tensor("out", [SEQ, D_MODEL], f32, kind="ExternalOutput").ap()

    with tile.TileContext(nc) as tc:
        with (
            tc.tile_pool(name="consts", bufs=1) as cpool,
            tc.tile_pool(name="weights", bufs=1) as wpool,
            tc.tile_pool(name="csn", bufs=2) as cspool,
            tc.tile_pool(name="big", bufs=1) as big,
            tc.tile_pool(name="atp", bufs=2) as atpool,
            tc.tile_pool(name="xstream", bufs=2) as xpool,
            tc.tile_pool(name="rope", bufs=2) as rpool,
            tc.tile_pool(name="vtile", bufs=1) as vpool,
            tc.tile_pool(name="pt", bufs=2) as ppool,
            tc.tile_pool(name="small", bufs=2) as spool,
            tc.tile_pool(name="dtile", bufs=2) as dpool,
            tc.tile_pool(name="rectile", bufs=2) as rcpool,
            tc.tile_pool(name="mm512", bufs=2, space="PSUM") as psA,
            tc.tile_pool(name="spsum", bufs=2, space="PSUM") as psS,
            tc.tile_pool(name="pvpsum", bufs=2, space="PSUM") as psV,
        ):
            # ---- constants into SBUF ----
            wq_sb = wpool.tile([128, KD * 512], f8, tag="wq", name="wq_sb")
            wk_sb = wpool.tile([128, KD * 512], f8, tag="wk", name="wk_sb")
            wv_sb = wpool.tile([128, KD * 512], f8, tag="wv", name="wv_sb")
            wq4 = wq_sb.rearrange("p (k n) -> p k n", k=KD)
            wk4 = wk_sb.rearrange("p (k n) -> p k n", k=KD)
            wv4 = wv_sb.rearrange("p (k n) -> p k n", k=KD)
            wq16 = wpool.tile([128, KD * 512], f16, tag="wq16",
                              name="wq16_sb")
            wk16 = wpool.tile([128, KD * 512], f16, tag="wk16",
                              name="wk16_sb")
            wv16a = wpool.tile([128, 4 * 512], f16, tag="wv16a",
                               name="wv16a_sb")
            wv16b = wpool.tile([128, 4 * 512], f16, tag="wv16b",
                               name="wv16b_sb")
            mask16 = cpool.tile([128, 128], f16, tag="mask16")
            nc.gpsimd.dma_start(mask16[:, :], mask_d[:, :])
            mask8 = cpool.tile([128, 128], f8, tag="mask8")
            nc.gpsimd.dma_start(mask8[:, :], mask8_d[:, :])
            ones_sb = cpool.tile([1, 64], f32r, tag="ones")
            nc.gpsimd.dma_start(ones_sb[:, :], ones_d[:, :])
            halves_sb = cpool.tile([65, 128], f32r, tag="halves")
            nc.gpsimd.dma_start(halves_sb[:, :], halves_d[:, :])

            # ---- persistent activations ----
            qt_t = [big.tile([128, SEQ], f16, tag=f"qt{c}", name=f"qt{c}")
                    for c in range(NCHUNK)]
            kt_t = [big.tile([128, SEQ], f16, tag=f"kt{c}", name=f"kt{c}")
                    for c in range(NCHUNK)]
            # V in SBUF, fp8, [128, head, kblock, 64+ones+15pad] (the pad
            # keeps the dual-fp8 ldweights 16B stride alignment)
            v_sb = vpool.tile([128, HPC * NSC * 80], f8, tag="vsb",
                              name="v_sb")
            v4 = v_sb.rearrange("p (h s n) -> p h s n", h=HPC, s=NSC)
            # 1.0 in e4m3 is 0x38; pad cols 65-79 stay uninitialized (the
            # junk PV output rows 65-79 are never read)
            nc.vector.memset(v4[:, :, :, 64:65].bitcast(u8), 0x38)

            # ---- projection pass tiles (x slice + rope tables) ----
            pass_tiles = {}

            def _pass_tiles(qt):
                if qt not in pass_tiles:
                    xq = xpool.tile([128, KD * 512], f8, tag="xq",
                                    name=f"xq_{qt}")
                    for k in range(KD):
                        nc.sync.dma_start(
                            xq[:, k * 512:(k + 1) * 512],
                            xT[k * 128:(k + 1) * 128, qt * 512:(qt + 1) * 512])
                    cs = cspool.tile([128, 512], f16, tag="cs",
                                     name=f"cs_{qt}")
                    sn = cspool.tile([128, 512], f16, tag="sn",
                                     name=f"sn_{qt}")
                    nc.gpsimd.dma_start(cs[:, :],
                                        cos_d[:, qt * 512:(qt + 1) * 512])
                    nc.gpsimd.dma_start(sn[:, :],
                                        sin_d[:, qt * 512:(qt + 1) * 512])
                    pass_tiles[qt] = (xq, cs, sn, None)
                return pass_tiles[qt]

            # ---- projection chain pieces (generator per chain) ----
            def qk_chain_pieces(qt, wi, c):
                """Q/K projection chain for (pass qt, W wi, chunk c): a list
                of closures to be emitted in order, interruptible between."""
                st = {}

                def mm_first():
                    xq = _pass_tiles(qt)[0]
                    ps = psA.tile([128, 512], f32, tag="mm512",
                                  name=f"qkps_{qt}_{wi}_{c}")
                    st["ps"] = ps
                    if qt == 0:
                        xa = xq[0].rearrange("p (k n) -> p k n", k=4)
                        w16 = (wq16, wk16)[wi]
                        for k in range(4):
                            nc.tensor.matmul(
                                ps[:, :],
                                w16[:, k * 512 + c * 128:
                                    k * 512 + (c + 1) * 128],
                                xa[:, k, :],
                                start=(k == 0), stop=False)
                        return
                    xq4 = xq.rearrange("p (k n) -> p k n", k=KD)
                    w4 = (wq4, wk4)[wi]
                    for j in range(2):
                        nc.tensor.matmul(
                            ps[:, :],
                            w4[:, 2 * j:2 * j + 2, c * 128:(c + 1) * 128],
                            xq4[:, 2 * j:2 * j + 2, :],
                            start=(j == 0), stop=False, perf_mode=DR)

                def mm_second():
                    xq = _pass_tiles(qt)[0]
                    ps = st["ps"]
                    if qt == 0:
                        xb = xq[1].rearrange("p (k n) -> p k n", k=4)
                        w16 = (wq16, wk16)[wi]
                        for k in range(4, KD):
                            nc.tensor.matmul(
                                ps[:, :],
                                w16[:, k * 512 + c * 128:
                                    k * 512 + (c + 1) * 128],
                                xb[:, k - 4, :],
                                start=False, stop=(k == KD - 1))
                        return
                    xq4 = xq.rearrange("p (k n) -> p k n", k=KD)
                    w4 = (wq4, wk4)[wi]
                    for j in range(2, 4):
                        nc.tensor.matmul(
                            ps[:, :],
                            w4[:, 2 * j:2 * j + 2, c * 128:(c + 1) * 128],
                            xq4[:, 2 * j:2 * j + 2, :],
                            start=False, stop=(j == 3), perf_mode=DR)

                def rope_tail():
                    _, cs, sn, _ = _pass_tiles(qt)
                    ps = st["ps"]
                    raw = rpool.tile([128, 512], f16, tag="raw",
                                     name=f"raw_{qt}_{wi}_{c}")
                    nc.vector.tensor_copy(raw[:, :], ps[:, :])
                    rot = rpool.tile([128, 512], f16, tag="rot",
                                     name=f"rot_{qt}_{wi}_{c}")
                    # swap 32-row blocks pairwise via SBUF->SBUF DMA on the
                    # gpsimd trigger queue
                    for q in range(4):
                        srow = (q // 2) * 64 + (1 - (q % 2)) * 32
                        nc.gpsimd.dma_start(rot[q * 32:(q + 1) * 32, :],
                                            raw[srow:srow + 32, :])
                    dst = (qt_t, kt_t)[wi]
                    dsl = dst[c][:, qt * 512:(qt + 1) * 512]
                    nc.vector.tensor_mul(dsl, raw[:, :], cs[:, :])
                    nc.vector.tensor_mul(rot[:, :], rot[:, :], sn[:, :])
                    nc.vector.tensor_add(dsl, dsl, rot[:, :])

                return [mm_first, mm_second, rope_tail]

            def v_chain_pieces(qt, scl):
                st = {}

                def mms():
                    xq = _pass_tiles(qt)[0]
                    ps = psA.tile([128, 512], f32, tag="mm512",
                                  name=f"vps_{qt}_{scl}")
                    st["ps"] = ps
                    if qt == 0:
                        for k in range(KD):
                            xh = (xq[0], xq[1])[k // 4].rearrange(
                                "p (k n) -> p k n", k=4)
                            wv = (wv16a, wv16b)[k // 4]
                            nc.tensor.matmul(
                                ps[:, :],
                                xh[:, k % 4, scl * 128:(scl + 1) * 128],
                                wv[:, (k % 4) * 512:(k % 4 + 1) * 512],
                                start=(k == 0), stop=(k == KD - 1))
                        return
                    xq4 = xq.rearrange("p (k n) -> p k n", k=KD)
                    for j in range(4):
                        nc.tensor.matmul(
                            ps[:, :],
                            xq4[:, 2 * j:2 * j + 2,
                                scl * 128:(scl + 1) * 128],
                            wv4[:, 2 * j:2 * j + 2, :],
                            start=(j == 0), stop=(j == 3), perf_mode=DR)

                def copy_out():
                    sc = qt * 4 + scl
                    v16v = _pass_tiles(qt)[3]
                    ps3 = st["ps"].rearrange("p (h n) -> p h n", h=HPC)
                    nc.vector.tensor_copy(v4[:, :, sc, 0:64], ps3[:, :, :])
                    if v16v is not None:
                        nc.vector.tensor_copy(v16v[:, :, scl, 0:64],
                                              ps3[:, :, :])

                return [mms, copy_out]

            def proj_pass_pieces(qt):
                pieces = []
                for wi in (0, 1):
                    for c in range(NCHUNK):
                        pieces += qk_chain_pieces(qt, wi, c)
                for scl in range(4):
                    pieces += v_chain_pieces(qt, scl)
                return pieces

            # ---- out-projection pieces ----
            at_by_t = {}

            def outproj_pieces(t):
                pieces = []
                for scl in range(4):
                    for nn in range(2):
                        st = {}

                        def mms(t=t, scl=scl, nn=nn, st=st):
                            at_c = at_by_t[t]
                            ps = psA.tile([128, 512], f32, tag="mm512",
                                          name=f"ops_{t}_{scl}_{nn}")
                            st["ps"] = ps
                            for kc in range(4):
                                wh = (wo_a, wo_b)[kc // 2]
                                base = (kc % 2) * D_MODEL + nn * 512
                                nc.tensor.matmul(
                                    ps[:, :],
                                    at_c[kc][:, scl * 128:(scl + 1) * 128],
                                    wh[:, base:base + 512],
                                    start=(kc == 0), stop=(kc == 3))

                        def tail(t=t, scl=scl, nn=nn, st=st):
                            osb = spool.tile([128, 512], f32, tag="osb",
                                             name=f"osb_{t}_{scl}_{nn}")
                            nc.scalar.copy(osb[:, :], st["ps"][:, :])
                            nc.sync.dma_start(
                                out_d[(t * 4 + scl) * 128:
                                      (t * 4 + scl + 1) * 128,
                                      nn * 512:(nn + 1) * 512],
                                osb[:, :])

                        pieces += [mms, tail]
                return pieces

            # ---- normalization pieces for tile t (after its recip) ----
            den_by_t = {}

            def recip_piece(t):
                # 1/den as exp(-ln(den)) on ACT; emitted as the first
                # normalize filler of tile t+1 so it doesn't sit between
                # the boundary exp instructions in the ACT queue
                denX, denY = den_by_t[t]
                for dn in (denX, denY):
                    nc.scalar.activation(dn[:, :], dn[:, :], Ln)
                    nc.scalar.activation(dn[:, :], dn[:, :], Exp,
                                         scale=-1.0)

            norm_state = {"inits": 0}

            def norm_pieces(t):
                pieces = []
                for c in range(NCHUNK):
                    def piece(t=t, c=c):
                        at_c = at_by_t[t]
                        denX, denY = den_by_t[t]
                        rec = rcpool.tile([65, 512], f32r, tag="rec",
                                          name=f"rec_{t}_{c}")
                        if norm_state["inits"] < 2:
                            # zero rows 1-63 once per pool slot; later
                            # generations inherit the zeros (only rows 0
                            # and 64 are ever rewritten), keeping the
                            # halves-matmul contraction junk-free
                            nc.vector.memset(
                                rec[0:64, :].bitcast(u8), 0)
                            norm_state["inits"] += 1
                        nc.vector.tensor_copy(rec[0:1, :],
                                              denX[32 * c:32 * c + 1, :])
                        nc.vector.tensor_copy(rec[64:65, :],
                                              denY[32 * c:32 * c + 1, :])
                        bps = psA.tile([128, 512], f32, tag="mm512",
                                       name=f"bps_{t}_{c}")
                        nc.tensor.matmul(bps[:, :], halves_sb[:, :],
                                         rec[:, :], start=True, stop=True)
                        nc.vector.tensor_mul(at_c[c][:, :], at_c[c][:, :],
                                             bps[:, :])
                    pieces.append(piece)
                return pieces

            # ---- attention step parts ----
            def emit_scores_diag(t, c, kb):
                """Scores + exp + mask for both heads of chunk c, diagonal
                key block kb (kb >= 4t). Returns the fp16 exp tile."""
                jd = kb - 4 * t
                lo = 128 * jd if jd > 0 else 0
                sps = psS.tile([128, 1024], f32, tag="sps",
                               name=f"sps_{t}_{c}_{kb}")
                nc.tensor.matmul(
                    sps[:, lo:512],
                    kt_t[c][0:64, kb * 128:(kb + 1) * 128],
                    qt_t[c][0:64, t * 512 + lo:(t + 1) * 512],
                    start=True, stop=True)
                nc.tensor.matmul(
                    sps[:, 512 + lo:1024],
                    kt_t[c][64:128, kb * 128:(kb + 1) * 128],
                    qt_t[c][64:128, t * 512 + lo:(t + 1) * 512],
                    start=True, stop=True)
                pt = ppool.tile([128, 1024], f16, tag="pt",
                                name=f"pt_{t}_{c}_{kb}")
                if lo == 0:
                    nc.scalar.activation(pt[:, :], sps[:, :], Exp,
                                         scale=EXP_SCALE)
                else:
                    sps2 = sps.rearrange("p (b n) -> p b n", b=2)
                    pt2 = pt.rearrange("p (b n) -> p b n", b=2)
                    nc.scalar.activation(pt2[:, :, lo:512],
                                         sps2[:, :, lo:512], Exp,
                                         scale=EXP_SCALE)
                for half in range(2):
                    base = half * 512 + lo
                    nc.vector.tensor_mul(pt[:, base:base + 128],
                                         pt[:, base:base + 128],
                                         mask16[:, 0:128])
                return pt

            def emit_scores_diag8(t, c, kb, pt8v):
                """Diagonal block for t>=1: scores (lo-trimmed), exp into
                the fp8 pair slot, zero the below-lo strip of odd slots,
                fp8 mask-mul on the 128-col diagonal."""
                jd = kb - 4 * t
                lo = 128 * jd if jd > 0 else 0
                slot = kb & 1
                sps = psS.tile([128, 1024], f32, tag="sps",
                               name=f"sps_{t}_{c}_{kb}")
                nc.tensor.matmul(
                    sps[:, lo:512],
                    kt_t[c][0:64, kb * 128:(kb + 1) * 128],
                    qt_t[c][0:64, t * 512 + lo:(t + 1) * 512],
                    start=True, stop=True)
                nc.tensor.matmul(
                    sps[:, 512 + lo:1024],
                    kt_t[c][64:128, kb * 128:(kb + 1) * 128],
                    qt_t[c][64:128, t * 512 + lo:(t + 1) * 512],
                    start=True, stop=True)
                sps2 = sps.rearrange("p (b n) -> p b n", b=2)
                if lo == 0:
                    nc.scalar.activation(pt8v[:, :, slot, :], sps2, Exp,
                                         scale=EXP_SCALE)
                else:
                    nc.scalar.activation(pt8v[:, :, slot, lo:512],
                                         sps2[:, :, lo:512], Exp,
                                         scale=EXP_SCALE)
                lo0 = 128 * (jd - 1) if jd >= 1 else 0
                if slot == 1 and lo > lo0:
                    nc.gpsimd.memset(
                        pt8v[:, :, 1, lo0:lo].bitcast(u8), 0)
                for half in range(2):
                    nc.vector.tensor_mul(pt8v[:, half, slot, lo:lo + 128],
                                         pt8v[:, half, slot, lo:lo + 128],
                                         mask8[:, :])

            def emit_scores_off(t, c, kb, pt8v):
                """Scores + exp into fp8 slot kb&1 for off-diagonal block."""
                sps = psS.tile([128, 1024], f32, tag="sps",
                               name=f"sps_{t}_{c}_{kb}")
                nc.tensor.matmul(
                    sps[:, 0:512],
                    kt_t[c][0:64, kb * 128:(kb + 1) * 128],
                    qt_t[c][0:64, t * 512:(t + 1) * 512],
                    start=True, stop=True)
                nc.tensor.matmul(
                    sps[:, 512:1024],
                    kt_t[c][64:128, kb * 128:(kb + 1) * 128],
                    qt_t[c][64:128, t * 512:(t + 1) * 512],
                    start=True, stop=True)
                spsh = sps.rearrange("p (h n) -> p h n", h=2)
                nc.scalar.activation(pt8v[:, :, kb & 1, :], spsh, Exp,
                                     scale=EXP_SCALE)

            def emit_pv_diag(t, c, kb, pt, pv_pair):
                nkb = 4 * t + 4
                jd = kb - 4 * t
                lo = 128 * jd if jd > 0 else 0
                pvA, pvB = pv_pair
                v16v = _pass_tiles(t)[3]
                nc.tensor.matmul(pvA[0:65, lo:512],
                                 v16v[:, 2 * c, kb - 4 * t, :],
                                 pt[:, lo:512],
                                 start=(kb == 0), stop=(kb == nkb - 1))
                nc.tensor.matmul(pvB[0:65, lo:512],
                                 v16v[:, 2 * c + 1, kb - 4 * t, :],
                                 pt[:, 512 + lo:1024],
                                 start=(kb == 0), stop=(kb == nkb - 1))

            def emit_pv_pair(t, c, kb0, pt8v, pv_pair, lo0=0, stop=False):
                pvA, pvB = pv_pair
                nc.tensor.matmul(pvA[:, lo0:512],
                                 v4[:, 2 * c, kb0:kb0 + 2, :],
                                 pt8v[:, 0, :, lo0:512],
                                 start=(kb0 == 0), stop=stop, perf_mode=DR)
                nc.tensor.matmul(pvB[:, lo0:512],
                                 v4[:, 2 * c + 1, kb0:kb0 + 2, :],
                                 pt8v[:, 1, :, lo0:512],
                                 start=(kb0 == 0), stop=stop, perf_mode=DR)

            def chunk_tail(t, c, pv_pair):
                """Move PV out, bank denominator rows (den row 32c).
                denX first on DVE: it is pvA's last DVE reader and gates the
                next chunk's pvA PSUM slot (tailA runs in parallel on ACT)."""
                pvA, pvB = pv_pair
                at_c = at_by_t[t]
                denX, denY = den_by_t[t]
                nc.vector.tensor_copy(denX[32 * c:32 * c + 1, :],
                                      pvA[64:65, :])
                nc.scalar.mul(at_c[c][0:64, :], pvA[0:64, :], 1.0 / 16.0)
                nc.vector.tensor_scalar_mul(at_c[c][64:128, :], pvB[0:64, :],
                                            1.0 / 16.0)
                nc.vector.tensor_copy(denY[32 * c:32 * c + 1, :],
                                      pvB[64:65, :])

            # ---- schedule ----
            with nc.named_scope("qkv_proj0"):
                # interleave W_Q[k] / x0[k] issue so matmul k can start as
                # soon as its two 256KB blocks land
                xq0a = xpool.tile([128, 4 * 512], f16, tag="xq0a",
                                  name="xq0a", bufs=1)
                xq0b = xpool.tile([128, 4 * 512], f16, tag="xq0b",
                                  name="xq0b", bufs=1)
                v16_0 = cspool.tile([128, HPC * 4 * 65], f16, tag="v16",
                                    name="v16_0")
                v16v0 = v16_0.rearrange("p (h s n) -> p h s n", h=HPC, s=4)
                nc.vector.memset(v16v0[:, :, :, 64:65], 1.0)
                for k in range(KD):
                    nc.sync.dma_start(wq16[:, k * 512:(k + 1) * 512],
                                      wq16T[k * 128:(k + 1) * 128, :])
                    xh = (xq0a, xq0b)[k // 4]
                    nc.sync.dma_start(
                        xh[:, (k % 4) * 512:(k % 4 + 1) * 512],
                        x16T[k * 128:(k + 1) * 128, :])
                cs0 = cspool.tile([128, 512], f16, tag="cs", name="cs_0")
                sn0 = cspool.tile([128, 512], f16, tag="sn", name="sn_0")
                nc.gpsimd.dma_start(cs0[:, :], cos_d[:, 0:512])
                nc.gpsimd.dma_start(sn0[:, :], sin_d[:, 0:512])
                pass_tiles[0] = ((xq0a, xq0b), cs0, sn0, v16v0)
                for k in range(KD):
                    nc.sync.dma_start(wk16[:, k * 512:(k + 1) * 512],
                                      wk16T[k * 128:(k + 1) * 128, :])
                for k in range(KD):
                    wvh = (wv16a, wv16b)[k // 4]
                    nc.sync.dma_start(wvh[:, (k % 4) * 512:(k % 4 + 1) * 512],
                                      wv16T[k * 128:(k + 1) * 128, :])
                for k in range(KD):
                    nc.sync.dma_start(wq_sb[:, k * 512:(k + 1) * 512],
                                      wqT[k * 128:(k + 1) * 128, :])
                    nc.sync.dma_start(wk_sb[:, k * 512:(k + 1) * 512],
                                      wkT[k * 128:(k + 1) * 128, :])
                    nc.sync.dma_start(wv_sb[:, k * 512:(k + 1) * 512],
                                      wvT[k * 128:(k + 1) * 128, :])
                for piece in proj_pass_pieces(0):
                    piece()

            wo_a = wpool.tile([128, 2 * D_MODEL], f16, tag="wo_a",
                              name="wo_a")
            wo_b = wpool.tile([128, 2 * D_MODEL], f16, tag="wo_b",
                              name="wo_b")

            def wo_dma_piece():
                for k in range(4):
                    wh = (wo_a, wo_b)[k // 2]
                    nc.sync.dma_start(
                        wh[:, (k % 2) * D_MODEL:(k % 2 + 1) * D_MODEL],
                        woT[k * 128:(k + 1) * 128, :])

            with nc.named_scope("attention"):
                pending_norm = []
                for t in range(NQT):
                    at_by_t[t] = [
                        atpool.tile([128, 512], f16, tag=f"at{c}",
                                    name=f"at{c}_{t}")
                        for c in range(NCHUNK)]
                    den_by_t[t] = (
                        dpool.tile([128, 512], f32, tag="denX",
                                   name=f"denX_{t}"),
                        dpool.tile([128, 512], f32, tag="denY",
                                   name=f"denY_{t}"))
                    nkb = 4 * t + 4
                    steps = [(c, kb) for c in range(NCHUNK)
                             for kb in range(nkb)]
                    # fillers: pass t+1 pieces early (their DMAs are
                    # issued eagerly now), normalization of t-1 mid-tile
                    # (its reciprocals need time on DVE), outproj(t-1) last
                    fillers = []
                    if t + 1 < NQT:
                        _pass_tiles(t + 1)
                        fillers += proj_pass_pieces(t + 1)
                    if t == 0:
                        fillers.append(wo_dma_piece)
                    fillers += pending_norm
                    pending_norm = []
                    if t >= 1:
                        fillers += outproj_pieces(t - 1)
                    fill_at = {}
                    for fi, piece in enumerate(fillers):
                        si = min(len(steps) - 1,
                                 (fi + 1) * len(steps) // (len(fillers) + 1))
                        fill_at.setdefault(si, []).append(piece)
                    pending_pv = None
                    pv_pair = None
                    pt8v = None
                    for si, (c, kb) in enumerate(steps):
                        diag = kb >= 4 * t
                        fp16diag = diag and t == 0
                        if kb % 2 == 0 and not fp16diag:
                            pt8 = ppool.tile([128, 2048], f8, tag="pt8",
                                             name=f"pt8_{t}_{c}_{kb}")
                            pt8v = pt8.rearrange(
                                "p (h k n) -> p h k n", h=2, k=2)
                        if fp16diag:
                            pt = emit_scores_diag(t, c, kb)
                        elif diag:
                            emit_scores_diag8(t, c, kb, pt8v)
                        else:
                            emit_scores_off(t, c, kb, pt8v)
                        if pending_pv is not None:
                            pending_pv()
                            pending_pv = None
                        if kb == 0:
                            pv_pair = (psV.tile([80, 512], f32, tag="pv",
                                                name=f"pvA_{t}_{c}"),
                                       psV.tile([80, 512], f32, tag="pv",
                                                name=f"pvB_{t}_{c}"))
                        if fp16diag:
                            def mk_pv(t=t, c=c, kb=kb, pt=pt,
                                      pv_pair=pv_pair):
                                emit_pv_diag(t, c, kb, pt, pv_pair)
                                if kb == nkb - 1:
                                    chunk_tail(t, c, pv_pair)
                            pending_pv = mk_pv
                        elif kb % 2 == 1:
                            jd0 = (kb - 1) - 4 * t
                            lo0 = 128 * jd0 if jd0 > 0 else 0
                            last = kb == nkb - 1
                            def mk_pv(t=t, c=c, kb0=kb - 1, pt8v=pt8v,
                                      pv_pair=pv_pair, lo0=lo0, last=last):
                                emit_pv_pair(t, c, kb0, pt8v, pv_pair,
                                             lo0=lo0, stop=last)
                                if last:
                                    chunk_tail(t, c, pv_pair)
                            pending_pv = mk_pv
                        for piece in fill_at.get(si, ()):
                            piece()
                    pending_pv()
                    pending_norm = [lambda t=t: recip_piece(t)]
                    pending_norm += norm_pieces(t)
                # tail: normalize + out-project the last tile
                for piece in pending_norm:
                    piece()
                for piece in outproj_pieces(NQT - 1):
                    piece()

    return nc


# ---------------------------------------------------------------------------
# host side
# ---------------------------------------------------------------------------
_PROG_CACHE = {}


def _get_program():
    if "nc" not in _PROG_CACHE:
        _install_axon_hooks()
        _install_drain_patch()
        _PROG_CACHE["nc"] = _build_program()
    return _PROG_CACHE["nc"]


def _prep_in_maps(inputs):
    x = np.asarray(inputs["x"], np.float32)
    pos = np.asarray(inputs["token_positions"]).astype(np.float32)
    WQ = np.asarray(inputs["W_Q"], np.float32)
    WK = np.asarray(inputs["W_K"], np.float32)
    WV = np.asarray(inputs["W_V"], np.float32)
    WO = np.asarray(inputs["W_O"], np.float32)

    # NeoX reorder of interleaved rope pairs, per head (rows of W_Q/W_K)
    perm = np.empty(D_MODEL, np.int64)
    for h in range(NUM_HEADS):
        b = h * HEAD_DIM
        perm[b:b + 32] = b + 2 * np.arange(32)
        perm[b + 32:b + 64] = b + 2 * np.arange(32) + 1
    # fp8 range rescale: W std is ~0.02, below the e4m3 normal range;
    # store 32x weights and fold 32*32 * HEAD_DIM**-0.5 into the exp scale
    f8 = ml_dtypes.float8_e4m3fn
    WQ32 = WQ[perm] * np.float32(32.0)
    WK32 = WK[perm] * np.float32(32.0)
    WV32 = WV * np.float32(32.0)
    WQp = WQ32.astype(f8)
    WKp = WK32.astype(f8)
    WVs = WV32.astype(f8)
    WQ16 = WQ32.astype(np.float16)
    WK16 = WK32.astype(np.float16)
    WV16 = WV32.astype(np.float16)
    x8 = x.astype(f8)
    x16 = x.astype(np.float16)

    # rope tables, mirroring the reference's float32 math
    j = np.arange(HEAD_DIM // 2, dtype=np.float32)
    inv_freq = np.power(np.float32(THETA),
                        (np.float32(-2.0) * j / np.float32(HEAD_DIM))
                        ).astype(np.float32)
    ang = pos[:, None] * inv_freq[None, :]          # (SEQ, 32) f32
    cos = np.cos(ang).astype(np.float32).T          # (32, SEQ)
    sin = np.sin(ang).astype(np.float32).T
    cos_t = np.ascontiguousarray(np.tile(cos, (4, 1))).astype(np.float16)
    sin_t = np.ascontiguousarray(
        np.concatenate([-sin, sin, -sin, sin], axis=0)).astype(np.float16)

    tri = (np.arange(128)[:, None] <= np.arange(128)[None, :])
    bigmask = tri.astype(np.float16)
    bigmask8 = tri.astype(ml_dtypes.float8_e4m3fn)
    ones64 = np.full((1, 64), 16.0 / 32.0, np.float32)
    halves65 = np.zeros((65, 128), np.float32)
    halves65[0, 0:64] = 16.0 / 32.0
    halves65[64, 64:128] = 16.0 / 32.0

    in_maps = []
    for core in range(N_CORES):
        b, g = core // 2, core % 2
        sl = slice(g * 512, (g + 1) * 512)
        in_maps.append({
            "xT": np.ascontiguousarray(x8[b].T),
            "wqT": np.ascontiguousarray(WQp[sl].T),
            "wkT": np.ascontiguousarray(WKp[sl].T),
            "wvT": np.ascontiguousarray(WVs[sl].T),
            "wq16T": np.ascontiguousarray(WQ16[sl].T),
            "wk16T": np.ascontiguousarray(WK16[sl].T),
            "wv16T": np.ascontiguousarray(WV16[sl].T),
            "x16T": np.ascontiguousarray(x16[b].T[:, 0:512]),
            "woT": np.ascontiguousarray(WO[:, sl].T.astype(np.float16)),
            "cos_t": cos_t,
            "sin_t": sin_t,
            "bigmask": bigmask,
            "bigmask8": bigmask8,
            "ones64": ones64,
            "halves65": halves65,
        })
    return in_maps


def kernel(**inputs):
    from concourse.bass_utils import run_bass_kernel_spmd

    nc = _get_program()
    if not _PROG_CACHE.get("waits_split"):
        _split_sync_waits(nc)
        _PROG_CACHE["waits_split"] = True
    in_maps = _prep_in_maps(inputs)
    trace = os.environ.get("BASS_KERNEL_TRACE") == "1"
    kw = {}
    if trace:
        kw = dict(trace=True, tmpdir=os.environ.get("BASS_KERNEL_TRACE_DIR"))
    res = run_bass_kernel_spmd(nc, in_maps, core_ids=list(range(N_CORES)), **kw)
    if trace:
        print(f"HW exec time: {res.exec_time_ns} ns "
              f"(mean {res.mean_exec_time_ns}, "
              f"max core {res.max_exec_time_core_id})")
        _PROG_CACHE["last_results"] = res

    out = np.empty((BATCH, SEQ, D_MODEL), np.float32)
    for b in range(BATCH):
        out[b] = res.results[2 * b]["out"] + res.results[2 * b + 1]["out"]
    return out


# revision 23
# speedup vs baseline: 1.2087x; 1.2087x over previous
"""Multi-head attention (RoPE + causal softmax) Trainium2 Bass kernel.

Sharding: 8 cores = 4 batches x 2 head-groups (tensor-parallel over heads).
Each core computes, for its (batch b, head-group g):
    Q/K/V projections for its 8 heads, RoPE, causal attention, and a
    partial output projection with its 512-row slice of W_O^T.
Host sums the two partial outputs per batch.

v13 (318us, from the 509us v2 baseline):
  - Attention inner loop software-pipelined: scores(kb+1) is emitted
    before PV(kb), so the ACT exp of step kb runs while the PE computes
    the next step's scores (the v2 loop serialized PE->ACT->PE per step).
  - fp8e4 DoubleRow (0.5 cyc/row) for the QKV projections of passes 1-3
    (x and 32x-rescaled W in fp8; 4 dual-k-tile matmuls per chain) and
    for off-diagonal PV key-block pairs (V padded to 80 cols for the
    dual-fp8 16B ldweights stride rule; exp writes fp8 P directly).
    Pass 0 and tile-0 attention stay fp16 so early tokens (small
    softmax fan-in, no error averaging) keep fp16-grade values; the
    diagonal blocks of tiles 1-3 also run as fp8 pairs (fp8 mask-mul
    on the exp output, u8-bitcast zeroing of below-diagonal strips).
    Scores stay fp16: dual-fp8 with 32-partition k-tiles measured
    ~600ns vs the model's 213ns, slower than the fp16 64-contraction
    form (each scores matmul is ~512cyc@1.2GHz + ~170ns fixed cost,
    capped by the 2KB PSUM bank; the PE never reaches its 2.4GHz
    p-state in this dependency-paced loop).
  - Scheduling: only proj pass 0 runs upfront; pass t+1 (early), then
    normalize(t-1), then outproj(t-1) spread as fine-grained fillers
    inside tile t; W_O load deferred into tile 0.
  - Normalization: per-tile denominators banked at 32-aligned rows,
    reciprocal as exp(-ln(x)) on the ACT engine (the 3.3us DVE
    reciprocal at tile boundaries was head-of-line-blocking the rope
    tails, which held PSUM slots and stalled the in-order PE queue),
    then one half-indicator [65,128] broadcast matmul + one in-place
    mul per chunk (rec pool slots keep rows 1-63 zeroed across
    generations so the contraction stays junk-free); at tiles fp16
    with a /16 guard scale against overflow.
  - Startup: pass-0 W/x DMAs issue k-interleaved so the first
    projection matmul starts after ~2 blocks, not the full 8MB.
"""

import os
import sys
import types

import ml_dtypes
import numpy as np

D_MODEL = 1024
NUM_HEADS = 16
HEAD_DIM = 64
THETA = 10000.0
BATCH = 4
SEQ = 2048
N_CORES = 8
HPC = 8          # heads per core
NCHUNK = HPC // 2  # 128-row chunks of the per-core 512 head dims
NQT = SEQ // 512   # 512-wide seq tiles
NSC = SEQ // 128   # 128-row seq chunks
KD = D_MODEL // 128  # contraction chunks for projections


# ---------------------------------------------------------------------------
# environment shims (axon container: missing antenv.axon_hooks; walrus here
# supports only 1 sync-wait per instruction)
# ---------------------------------------------------------------------------
def _install_axon_hooks():
    import antenv

    if hasattr(antenv, "axon_hooks"):
        return
    mod = types.ModuleType("antenv.axon_hooks")
    _hook = [None]
    mod.set_axon_ntff_profile_hook = lambda h: _hook.__setitem__(0, h)
    mod.get_axon_ntff_profile_hook = lambda: _hook[0]
    sys.modules["antenv.axon_hooks"] = mod
    antenv.axon_hooks = mod
    try:
        from trn_agent_boot.trn_boot import _ntff_profile_via_ctypes

        mod.set_axon_ntff_profile_hook(
            _ntff_profile_via_ctypes("/opt/axon/libaxon_pjrt.so")
        )
    except Exception:
        pass


def _install_drain_patch():
    import concourse.mybir as mybir
    import concourse.tile as tilemod

    if getattr(tilemod.TileContext, "_drain_patch_installed", False):
        return

    def _drain_and_barrier(self, tick_clock, wait_clock):
        carrier = self.nc.sync.nop(nofuse=True)
        wait_clock.add_sem_waits(
            carrier.ins, tilemod.ScopedClock({None: tick_clock.global_clock})
        )
        si = carrier.ins.sync_info
        if si is not None and si.on_wait and len(si.on_wait) > 1:
            waits = list(si.on_wait)
            carrier.ins.sync_info = mybir.SyncInfo(
                on_wait=[waits[0]], on_update=list(si.on_update or [])
            )
            for w in waits[1:]:
                nop = self.nc.sync.nop(nofuse=True)
                nop.ins.sync_info = mybir.SyncInfo(on_wait=[w], on_update=[])
        self.nc.sync.drain()

        self.nc.all_engine_barrier()
        assert self.sems is not None
        popped = self.nc._tile_sem_poison_stack.pop()
        assert popped is self._sem_poison
        self.nc.clear_and_free_semaphores(list(self.sems.allocated().values()))
        self.nc.all_engine_barrier()

    tilemod.TileContext._drain_and_barrier = _drain_and_barrier
    tilemod.TileContext._drain_patch_installed = True


def _split_sync_waits(nc, max_waits=1):
    """Hoist excess per-instruction sem waits onto same-engine NoOps."""
    import concourse.mybir as mybir

    n_added = 0
    for fn in nc.m.functions:
        for bb in fn.blocks:
            insts = bb.instructions
            new_list = []
            changed = False
            for inst in insts:
                si = inst.sync_info
                waits = list(si.on_wait) if si is not None and si.on_wait else []
                if (
                    len(waits) > max_waits
                    and inst.engine != mybir.EngineType.Unassigned
                ):
                    keep = waits[-max_waits:]
                    extra = waits[:-max_waits]
                    while extra:
                        chunk, extra = extra[:max_waits], extra[max_waits:]
                        nop = mybir.InstNoOp(
                            name=f"I-waitsplit-{n_added}", ins=[], outs=[]
                        )
                        nop.engine = inst.engine
                        nop.bass_nofuse = True
                        nop.sync_info = mybir.SyncInfo(on_wait=chunk, on_update=[])
                        new_list.append(nop)
                        n_added += 1
                    inst.sync_info = mybir.SyncInfo(
                        on_wait=keep, on_update=list(si.on_update or [])
                    )
                    changed = True
                new_list.append(inst)
            if changed:
                bb.instructions = new_list
    return n_added


# ---------------------------------------------------------------------------
# device program
# ---------------------------------------------------------------------------
def _build_program():
    import concourse.bass as bass
    import concourse.mybir as mybir
    import concourse.tile as tile

    f32 = mybir.dt.float32
    f32r = mybir.dt.float32r
    f16 = mybir.dt.float16
    f8 = mybir.dt.float8e4
    u8 = mybir.dt.uint8
    DR = mybir.MatmulPerfMode.DoubleRow
    Exp = mybir.ActivationFunctionType.Exp
    Ln = mybir.ActivationFunctionType.Ln
    EXP_SCALE = 2.0 ** -13  # folds the 1/sqrt(64) score scale and the
    # 32x fp8-range rescale of W_Q/W_K (32*32*8 = 8192)

    nc = bass.Bass("TRN2", target_bir_lowering=False, debug=False,
                   num_devices=N_CORES)

    xT = nc.dram_tensor("xT", [D_MODEL, SEQ], f8, kind="ExternalInput").ap()
    wqT = nc.dram_tensor("wqT", [D_MODEL, 512], f8, kind="ExternalInput").ap()
    wkT = nc.dram_tensor("wkT", [D_MODEL, 512], f8, kind="ExternalInput").ap()
    wvT = nc.dram_tensor("wvT", [D_MODEL, 512], f8, kind="ExternalInput").ap()
    wq16T = nc.dram_tensor("wq16T", [D_MODEL, 512], f16,
                           kind="ExternalInput").ap()
    wk16T = nc.dram_tensor("wk16T", [D_MODEL, 512], f16,
                           kind="ExternalInput").ap()
    wv16T = nc.dram_tensor("wv16T", [D_MODEL, 512], f16,
                           kind="ExternalInput").ap()
    x16T = nc.dram_tensor("x16T", [D_MODEL, 512], f16,
                          kind="ExternalInput").ap()
    woT = nc.dram_tensor("woT", [512, D_MODEL], f16, kind="ExternalInput").ap()
    cos_d = nc.dram_tensor("cos_t", [128, SEQ], f16, kind="ExternalInput").ap()
    sin_d = nc.dram_tensor("sin_t", [128, SEQ], f16, kind="ExternalInput").ap()
    mask_d = nc.dram_tensor("bigmask", [128, 128], f16, kind="ExternalInput").ap()
    mask8_d = nc.dram_tensor("bigmask8", [128, 128], f8,
                             kind="ExternalInput").ap()
    ones_d = nc.dram_tensor("ones64", [1, 64], f32r, kind="ExternalInput").ap()
    halves_d = nc.dram_tensor("halves65", [65, 128], f32r,
                              kind="ExternalInput").ap()
    out_d = nc.dram_tensor("out", [SEQ, D_MODEL], f32, kind="ExternalOutput").ap()

    with tile.TileContext(nc) as tc:
        with (
            tc.tile_pool(name="consts", bufs=1) as cpool,
            tc.tile_pool(name="weights", bufs=1) as wpool,
            tc.tile_pool(name="csn", bufs=2) as cspool,
            tc.tile_pool(name="big", bufs=1) as big,
            tc.tile_pool(name="atp", bufs=2) as atpool,
            tc.tile_pool(name="xstream", bufs=2) as xpool,
            tc.tile_pool(name="rope", bufs=2) as rpool,
            tc.tile_pool(name="vtile", bufs=1) as vpool,
            tc.tile_pool(name="pt", bufs=2) as ppool,
            tc.tile_pool(name="small", bufs=2) as spool,
            tc.tile_pool(name="dtile", bufs=2) as dpool,
            tc.tile_pool(name="rectile", bufs=2) as rcpool,
            tc.tile_pool(name="mm512", bufs=2, space="PSUM") as psA,
            tc.tile_pool(name="spsum", bufs=2, space="PSUM") as psS,
            tc.tile_pool(name="pvpsum", bufs=2, space="PSUM") as psV,
        ):
            # ---- constants into SBUF ----
            wq_sb = wpool.tile([128, KD * 512], f8, tag="wq", name="wq_sb")
            wk_sb = wpool.tile([128, KD * 512], f8, tag="wk", name="wk_sb")
            wv_sb = wpool.tile([128, KD * 512], f8, tag="wv", name="wv_sb")
            wq4 = wq_sb.rearrange("p (k n) -> p k n", k=KD)
            wk4 = wk_sb.rearrange("p (k n) -> p k n", k=KD)
            wv4 = wv_sb.rearrange("p (k n) -> p k n", k=KD)
            wq16 = wpool.tile([128, KD * 512], f16, tag="wq16",
                              name="wq16_sb")
            wk16 = wpool.tile([128, KD * 512], f16, tag="wk16",
                              name="wk16_sb")
            wv16a = wpool.tile([128, 4 * 512], f16, tag="wv16a",
                               name="wv16a_sb")
            wv16b = wpool.tile([128, 4 * 512], f16, tag="wv16b",
                               name="wv16b_sb")
            mask16 = cpool.tile([128, 128], f16, tag="mask16")
            nc.gpsimd.dma_start(mask16[:, :], mask_d[:, :])
            mask8 = cpool.tile([128, 128], f8, tag="mask8")
            nc.gpsimd.dma_start(mask8[:, :], mask8_d[:, :])
            ones_sb = cpool.tile([1, 64], f32r, tag="ones")
            nc.gpsimd.dma_start(ones_sb[:, :], ones_d[:, :])
            halves_sb = cpool.tile([65, 128], f32r, tag="halves")
            nc.gpsimd.dma_start(halves_sb[:, :], halves_d[:, :])

            # ---- persistent activations ----
            qt_t = [big.tile([128, SEQ], f16, tag=f"qt{c}", name=f"qt{c}")
                    for c in range(NCHUNK)]
            kt_t = [big.tile([128, SEQ], f16, tag=f"kt{c}", name=f"kt{c}")
                    for c in range(NCHUNK)]
            # V in SBUF, fp8, [128, head, kblock, 64+ones+15pad] (the pad
            # keeps the dual-fp8 ldweights 16B stride alignment)
            v_sb = vpool.tile([128, HPC * NSC * 80], f8, tag="vsb",
                              name="v_sb")
            v4 = v_sb.rearrange("p (h s n) -> p h s n", h=HPC, s=NSC)
            # 1.0 in e4m3 is 0x38; pad cols 65-79 stay uninitialized (the
            # junk PV output rows 65-79 are never read)
            nc.vector.memset(v4[:, :, :, 64:65].bitcast(u8), 0x38)

            # ---- projection pass tiles (x slice + rope tables) ----
            pass_tiles = {}

            def _pass_tiles(qt):
                if qt not in pass_tiles:
                    xq = xpool.tile([128, KD * 512], f8, tag="xq",
                                    name=f"xq_{qt}")
                    for k in range(KD):
                        nc.sync.dma_start(
                            xq[:, k * 512:(k + 1) * 512],
                            xT[k * 128:(k + 1) * 128, qt * 512:(qt + 1) * 512])
                    cs = cspool.tile([128, 512], f16, tag="cs",
                                     name=f"cs_{qt}")
                    sn = cspool.tile([128, 512], f16, tag="sn",
                                     name=f"sn_{qt}")
                    nc.gpsimd.dma_start(cs[:, :],
                                        cos_d[:, qt * 512:(qt + 1) * 512])
                    nc.gpsimd.dma_start(sn[:, :],
                                        sin_d[:, qt * 512:(qt + 1) * 512])
                    pass_tiles[qt] = (xq, cs, sn, None)
                return pass_tiles[qt]

            # ---- projection chain pieces (generator per chain) ----
            def qk_chain_pieces(qt, wi, c):
                """Q/K projection chain for (pass qt, W wi, chunk c): a list
                of closures to be emitted in order, interruptible between."""
                st = {}

                def mm_first():
                    xq = _pass_tiles(qt)[0]
                    ps = psA.tile([128, 512], f32, tag="mm512",
                                  name=f"qkps_{qt}_{wi}_{c}")
                    st["ps"] = ps
                    if qt == 0:
                        xa = xq[0].rearrange("p (k n) -> p k n", k=4)
                        w16 = (wq16, wk16)[wi]
                        for k in range(4):
                            nc.tensor.matmul(
                                ps[:, :],
                                w16[:, k * 512 + c * 128:
                                    k * 512 + (c + 1) * 128],
                                xa[:, k, :],
                                start=(k == 0), stop=False)
                        return
                    xq4 = xq.rearrange("p (k n) -> p k n", k=KD)
                    w4 = (wq4, wk4)[wi]
                    for j in range(2):
                        nc.tensor.matmul(
                            ps[:, :],
                            w4[:, 2 * j:2 * j + 2, c * 128:(c + 1) * 128],
                            xq4[:, 2 * j:2 * j + 2, :],
                            start=(j == 0), stop=False, perf_mode=DR)

                def mm_second():
                    xq = _pass_tiles(qt)[0]
                    ps = st["ps"]
                    if qt == 0:
                        xb = xq[1].rearrange("p (k n) -> p k n", k=4)
                        w16 = (wq16, wk16)[wi]
                        for k in range(4, KD):
                            nc.tensor.matmul(
                                ps[:, :],
                                w16[:, k * 512 + c * 128:
                                    k * 512 + (c + 1) * 128],
                                xb[:, k - 4, :],
                                start=False, stop=(k == KD - 1))
                        return
                    xq4 = xq.rearrange("p (k n) -> p k n", k=KD)
                    w4 = (wq4, wk4)[wi]
                    for j in range(2, 4):
                        nc.tensor.matmul(
                            ps[:, :],
                            w4[:, 2 * j:2 * j + 2, c * 128:(c + 1) * 128],
                            xq4[:, 2 * j:2 * j + 2, :],
                            start=False, stop=(j == 3), perf_mode=DR)

                def rope_tail():
                    _, cs, sn, _ = _pass_tiles(qt)
                    ps = st["ps"]
                    raw = rpool.tile([128, 512], f16, tag="raw",
                                     name=f"raw_{qt}_{wi}_{c}")
                    nc.vector.tensor_copy(raw[:, :], ps[:, :])
                    rot = rpool.tile([128, 512], f16, tag="rot",
                                     name=f"rot_{qt}_{wi}_{c}")
                    # swap 32-row blocks pairwise via SBUF->SBUF DMA on the
                    # gpsimd trigger queue
                    for q in range(4):
                        srow = (q // 2) * 64 + (1 - (q % 2)) * 32
                        nc.gpsimd.dma_start(rot[q * 32:(q + 1) * 32, :],
                                            raw[srow:srow + 32, :])
                    dst = (qt_t, kt_t)[wi]
                    dsl = dst[c][:, qt * 512:(qt + 1) * 512]
                    nc.vector.tensor_mul(dsl, raw[:, :], cs[:, :])
                    nc.vector.tensor_mul(rot[:, :], rot[:, :], sn[:, :])
                    nc.vector.tensor_add(dsl, dsl, rot[:, :])

                return [mm_first, mm_second, rope_tail]

            def v_chain_pieces(qt, scl):
                st = {}

                def mms():
                    xq = _pass_tiles(qt)[0]
                    ps = psA.tile([128, 512], f32, tag="mm512",
                                  name=f"vps_{qt}_{scl}")
                    st["ps"] = ps
                    if qt == 0:
                        for k in range(KD):
                            xh = (xq[0], xq[1])[k // 4].rearrange(
                                "p (k n) -> p k n", k=4)
                            wv = (wv16a, wv16b)[k // 4]
                            nc.tensor.matmul(
                                ps[:, :],
                                xh[:, k % 4, scl * 128:(scl + 1) * 128],
                                wv[:, (k % 4) * 512:(k % 4 + 1) * 512],
                                start=(k == 0), stop=(k == KD - 1))
                        return
                    xq4 = xq.rearrange("p (k n) -> p k n", k=KD)
                    for j in range(4):
                        nc.tensor.matmul(
                            ps[:, :],
                            xq4[:, 2 * j:2 * j + 2,
                                scl * 128:(scl + 1) * 128],
                            wv4[:, 2 * j:2 * j + 2, :],
                            start=(j == 0), stop=(j == 3), perf_mode=DR)

                def copy_out():
                    sc = qt * 4 + scl
                    v16v = _pass_tiles(qt)[3]
                    ps3 = st["ps"].rearrange("p (h n) -> p h n", h=HPC)
                    nc.vector.tensor_copy(v4[:, :, sc, 0:64], ps3[:, :, :])
                    if v16v is not None:
                        nc.vector.tensor_copy(v16v[:, :, scl, 0:64],
                                              ps3[:, :, :])

                return [mms, copy_out]

            def proj_pass_pieces(qt):
                pieces = []
                for wi in (0, 1):
                    for c in range(NCHUNK):
                        pieces += qk_chain_pieces(qt, wi, c)
                for scl in range(4):
                    pieces += v_chain_pieces(qt, scl)
                return pieces

            # ---- out-projection pieces ----
            at_by_t = {}

            def outproj_pieces(t):
                pieces = []
                for scl in range(4):
                    for nn in range(2):
                        st = {}

                        def mms(t=t, scl=scl, nn=nn, st=st):
                            at_c = at_by_t[t]
                            ps = psA.tile([128, 512], f32, tag="mm512",
                                          name=f"ops_{t}_{scl}_{nn}")
                            st["ps"] = ps
                            for kc in range(4):
                                wh = (wo_a, wo_b)[kc // 2]
                                base = (kc % 2) * D_MODEL + nn * 512
                                nc.tensor.matmul(
                                    ps[:, :],
                                    at_c[kc][:, scl * 128:(scl + 1) * 128],
                                    wh[:, base:base + 512],
                                    start=(kc == 0), stop=(kc == 3))

                        def tail(t=t, scl=scl, nn=nn, st=st):
                            osb = spool.tile([128, 512], f32, tag="osb",
                                             name=f"osb_{t}_{scl}_{nn}",
                                             bufs=3)
                            nc.scalar.copy(osb[:, :], st["ps"][:, :])
                            nc.sync.dma_start(
                                out_d[(t * 4 + scl) * 128:
                                      (t * 4 + scl + 1) * 128,
                                      nn * 512:(nn + 1) * 512],
                                osb[:, :])

                        pieces += [mms, tail]
                return pieces

            # ---- normalization pieces for tile t (after its recip) ----
            den_by_t = {}

            def recip_piece(t):
                # 1/den as exp(-ln(den)) on ACT; emitted as the first
                # normalize filler of tile t+1 so it doesn't sit between
                # the boundary exp instructions in the ACT queue
                denX, denY = den_by_t[t]
                for dn in (denX, denY):
                    nc.scalar.activation(dn[:, :], dn[:, :], Ln)
                    nc.scalar.activation(dn[:, :], dn[:, :], Exp,
                                         scale=-1.0)

            norm_state = {"inits": 0}

            def norm_pieces(t):
                pieces = []
                for c in range(NCHUNK):
                    def piece(t=t, c=c):
                        at_c = at_by_t[t]
                        denX, denY = den_by_t[t]
                        rec = rcpool.tile([65, 512], f32r, tag="rec",
                                          name=f"rec_{t}_{c}")
                        if norm_state["inits"] < 2:
                            # zero rows 1-63 once per pool slot; later
                            # generations inherit the zeros (only rows 0
                            # and 64 are ever rewritten), keeping the
                            # halves-matmul contraction junk-free
                            nc.vector.memset(
                                rec[0:64, :].bitcast(u8), 0)
                            norm_state["inits"] += 1
                        nc.vector.tensor_copy(rec[0:1, :],
                                              denX[32 * c:32 * c + 1, :])
                        nc.vector.tensor_copy(rec[64:65, :],
                                              denY[32 * c:32 * c + 1, :])
                        bps = psA.tile([128, 512], f32, tag="mm512",
                                       name=f"bps_{t}_{c}")
                        nc.tensor.matmul(bps[:, :], halves_sb[:, :],
                                         rec[:, :], start=True, stop=True)
                        nc.vector.tensor_mul(at_c[c][:, :], at_c[c][:, :],
                                             bps[:, :])
                    pieces.append(piece)
                return pieces

            # ---- attention step parts ----
            def emit_scores_diag(t, c, kb):
                """Scores + exp + mask for both heads of chunk c, diagonal
                key block kb (kb >= 4t). Returns the fp16 exp tile."""
                jd = kb - 4 * t
                lo = 128 * jd if jd > 0 else 0
                sps = psS.tile([128, 1024], f32, tag="sps",
                               name=f"sps_{t}_{c}_{kb}")
                nc.tensor.matmul(
                    sps[:, lo:512],
                    kt_t[c][0:64, kb * 128:(kb + 1) * 128],
                    qt_t[c][0:64, t * 512 + lo:(t + 1) * 512],
                    start=True, stop=True)
                nc.tensor.matmul(
                    sps[:, 512 + lo:1024],
                    kt_t[c][64:128, kb * 128:(kb + 1) * 128],
                    qt_t[c][64:128, t * 512 + lo:(t + 1) * 512],
                    start=True, stop=True)
                pt = ppool.tile([128, 1024], f16, tag="pt",
                                name=f"pt_{t}_{c}_{kb}")
                if lo == 0:
                    nc.scalar.activation(pt[:, :], sps[:, :], Exp,
                                         scale=EXP_SCALE)
                else:
                    sps2 = sps.rearrange("p (b n) -> p b n", b=2)
                    pt2 = pt.rearrange("p (b n) -> p b n", b=2)
                    nc.scalar.activation(pt2[:, :, lo:512],
                                         sps2[:, :, lo:512], Exp,
                                         scale=EXP_SCALE)
                for half in range(2):
                    base = half * 512 + lo
                    nc.vector.tensor_mul(pt[:, base:base + 128],
                                         pt[:, base:base + 128],
                                         mask16[:, 0:128])
                return pt

            def emit_scores_diag8(t, c, kb, pt8v):
                """Diagonal block for t>=1: scores (lo-trimmed), exp into
                the fp8 pair slot, zero the below-lo strip of odd slots,
                fp8 mask-mul on the 128-col diagonal."""
                jd = kb - 4 * t
                lo = 128 * jd if jd > 0 else 0
                slot = kb & 1
                sps = psS.tile([128, 1024], f32, tag="sps",
                               name=f"sps_{t}_{c}_{kb}")
                nc.tensor.matmul(
                    sps[:, lo:512],
                    kt_t[c][0:64, kb * 128:(kb + 1) * 128],
                    qt_t[c][0:64, t * 512 + lo:(t + 1) * 512],
                    start=True, stop=True)
                nc.tensor.matmul(
                    sps[:, 512 + lo:1024],
                    kt_t[c][64:128, kb * 128:(kb + 1) * 128],
                    qt_t[c][64:128, t * 512 + lo:(t + 1) * 512],
                    start=True, stop=True)
                sps2 = sps.rearrange("p (b n) -> p b n", b=2)
                if lo == 0:
                    nc.scalar.activation(pt8v[:, :, slot, :], sps2, Exp,
                                         scale=EXP_SCALE)
                else:
                    nc.scalar.activation(pt8v[:, :, slot, lo:512],
                                         sps2[:, :, lo:512], Exp,
                                         scale=EXP_SCALE)
                lo0 = 128 * (jd - 1) if jd >= 1 else 0
                if slot == 1 and lo > lo0:
                    nc.gpsimd.memset(
                        pt8v[:, :, 1, lo0:lo].bitcast(u8), 0)
                for half in range(2):
                    nc.vector.tensor_mul(pt8v[:, half, slot, lo:lo + 128],
                                         pt8v[:, half, slot, lo:lo + 128],
                                         mask8[:, :])

            def emit_scores_off(t, c, kb, pt8v):
                """Scores + exp into fp8 slot kb&1 for off-diagonal block."""
                sps = psS.tile([128, 1024], f32, tag="sps",
                               name=f"sps_{t}_{c}_{kb}")
                nc.tensor.matmul(
                    sps[:, 0:512],
                    kt_t[c][0:64, kb * 128:(kb + 1) * 128],
                    qt_t[c][0:64, t * 512:(t + 1) * 512],
                    start=True, stop=True)
                nc.tensor.matmul(
                    sps[:, 512:1024],
                    kt_t[c][64:128, kb * 128:(kb + 1) * 128],
                    qt_t[c][64:128, t * 512:(t + 1) * 512],
                    start=True, stop=True)
                spsh = sps.rearrange("p (h n) -> p h n", h=2)
                nc.scalar.activation(pt8v[:, :, kb & 1, :], spsh, Exp,
                                     scale=EXP_SCALE)

            def emit_pv_diag(t, c, kb, pt, pv_pair):
                nkb = 4 * t + 4
                jd = kb - 4 * t
                lo = 128 * jd if jd > 0 else 0
                pvA, pvB = pv_pair
                v16v = _pass_tiles(t)[3]
                nc.tensor.matmul(pvA[0:65, lo:512],
                                 v16v[:, 2 * c, kb - 4 * t, :],
                                 pt[:, lo:512],
                                 start=(kb == 0), stop=(kb == nkb - 1))
                nc.tensor.matmul(pvB[0:65, lo:512],
                                 v16v[:, 2 * c + 1, kb - 4 * t, :],
                                 pt[:, 512 + lo:1024],
                                 start=(kb == 0), stop=(kb == nkb - 1))

            def emit_pv_pair(t, c, kb0, pt8v, pv_pair, lo0=0, stop=False):
                pvA, pvB = pv_pair
                nc.tensor.matmul(pvA[:, lo0:512],
                                 v4[:, 2 * c, kb0:kb0 + 2, :],
                                 pt8v[:, 0, :, lo0:512],
                                 start=(kb0 == 0), stop=stop, perf_mode=DR)
                nc.tensor.matmul(pvB[:, lo0:512],
                                 v4[:, 2 * c + 1, kb0:kb0 + 2, :],
                                 pt8v[:, 1, :, lo0:512],
                                 start=(kb0 == 0), stop=stop, perf_mode=DR)

            def chunk_tail(t, c, pv_pair):
                """Move PV out, bank denominator rows (den row 32c).
                denX first on DVE: it is pvA's last DVE reader and gates the
                next chunk's pvA PSUM slot (tailA runs in parallel on ACT)."""
                pvA, pvB = pv_pair
                at_c = at_by_t[t]
                denX, denY = den_by_t[t]
                nc.vector.tensor_copy(denX[32 * c:32 * c + 1, :],
                                      pvA[64:65, :])
                nc.scalar.mul(at_c[c][0:64, :], pvA[0:64, :], 1.0 / 16.0)
                nc.vector.tensor_scalar_mul(at_c[c][64:128, :], pvB[0:64, :],
                                            1.0 / 16.0)
                nc.vector.tensor_copy(denY[32 * c:32 * c + 1, :],
                                      pvB[64:65, :])

            # ---- schedule ----
            with nc.named_scope("qkv_proj0"):
                # interleave W_Q[k] / x0[k] issue so matmul k can start as
                # soon as its two 256KB blocks land
                xq0a = xpool.tile([128, 4 * 512], f16, tag="xq0a",
                                  name="xq0a", bufs=1)
                xq0b = xpool.tile([128, 4 * 512], f16, tag="xq0b",
                                  name="xq0b", bufs=1)
                v16_0 = cspool.tile([128, HPC * 4 * 65], f16, tag="v16",
                                    name="v16_0")
                v16v0 = v16_0.rearrange("p (h s n) -> p h s n", h=HPC, s=4)
                nc.vector.memset(v16v0[:, :, :, 64:65], 1.0)
                for k in range(KD):
                    nc.sync.dma_start(wq16[:, k * 512:(k + 1) * 512],
                                      wq16T[k * 128:(k + 1) * 128, :])
                    xh = (xq0a, xq0b)[k // 4]
                    nc.sync.dma_start(
                        xh[:, (k % 4) * 512:(k % 4 + 1) * 512],
                        x16T[k * 128:(k + 1) * 128, :])
                cs0 = cspool.tile([128, 512], f16, tag="cs", name="cs_0")
                sn0 = cspool.tile([128, 512], f16, tag="sn", name="sn_0")
                nc.gpsimd.dma_start(cs0[:, :], cos_d[:, 0:512])
                nc.gpsimd.dma_start(sn0[:, :], sin_d[:, 0:512])
                pass_tiles[0] = ((xq0a, xq0b), cs0, sn0, v16v0)
                for k in range(KD):
                    nc.sync.dma_start(wk16[:, k * 512:(k + 1) * 512],
                                      wk16T[k * 128:(k + 1) * 128, :])
                for k in range(KD):
                    wvh = (wv16a, wv16b)[k // 4]
                    nc.sync.dma_start(wvh[:, (k % 4) * 512:(k % 4 + 1) * 512],
                                      wv16T[k * 128:(k + 1) * 128, :])
                for k in range(KD):
                    nc.sync.dma_start(wq_sb[:, k * 512:(k + 1) * 512],
                                      wqT[k * 128:(k + 1) * 128, :])
                    nc.sync.dma_start(wk_sb[:, k * 512:(k + 1) * 512],
                                      wkT[k * 128:(k + 1) * 128, :])
                    nc.sync.dma_start(wv_sb[:, k * 512:(k + 1) * 512],
                                      wvT[k * 128:(k + 1) * 128, :])
                for piece in proj_pass_pieces(0):
                    piece()

            wo_a = wpool.tile([128, 2 * D_MODEL], f16, tag="wo_a",
                              name="wo_a")
            wo_b = wpool.tile([128, 2 * D_MODEL], f16, tag="wo_b",
                              name="wo_b")

            def wo_dma_piece():
                for k in range(4):
                    wh = (wo_a, wo_b)[k // 2]
                    nc.sync.dma_start(
                        wh[:, (k % 2) * D_MODEL:(k % 2 + 1) * D_MODEL],
                        woT[k * 128:(k + 1) * 128, :])

            with nc.named_scope("attention"):
                pending_norm = []
                for t in range(NQT):
                    at_by_t[t] = [
                        atpool.tile([128, 512], f16, tag=f"at{c}",
                                    name=f"at{c}_{t}")
                        for c in range(NCHUNK)]
                    den_by_t[t] = (
                        dpool.tile([128, 512], f32, tag="denX",
                                   name=f"denX_{t}"),
                        dpool.tile([128, 512], f32, tag="denY",
                                   name=f"denY_{t}"))
                    nkb = 4 * t + 4
                    steps = [(c, kb) for c in range(NCHUNK)
                             for kb in range(nkb)]
                    # fillers: pass t+1 pieces early (their DMAs are
                    # issued eagerly now), normalization of t-1 mid-tile
                    # (its reciprocals need time on DVE), outproj(t-1) last
                    fillers = []
                    if t + 1 < NQT:
                        _pass_tiles(t + 1)
                        fillers += proj_pass_pieces(t + 1)
                    if t == 0:
                        fillers.append(wo_dma_piece)
                    fillers += pending_norm
                    pending_norm = []
                    if t >= 1:
                        fillers += outproj_pieces(t - 1)
                    fill_at = {}
                    for fi, piece in enumerate(fillers):
                        si = min(len(steps) - 1,
                                 (fi + 1) * len(steps) // (len(fillers) + 1))
                        fill_at.setdefault(si, []).append(piece)
                    pending_pv = None
                    pv_pair = None
                    pt8v = None
                    for si, (c, kb) in enumerate(steps):
                        diag = kb >= 4 * t
                        fp16diag = diag and t == 0
                        if kb % 2 == 0 and not fp16diag:
                            pt8 = ppool.tile([128, 2048], f8, tag="pt8",
                                             name=f"pt8_{t}_{c}_{kb}",
                                             bufs=3)
                            pt8v = pt8.rearrange(
                                "p (h k n) -> p h k n", h=2, k=2)
                        if fp16diag:
                            pt = emit_scores_diag(t, c, kb)
                        elif diag:
                            emit_scores_diag8(t, c, kb, pt8v)
                        else:
                            emit_scores_off(t, c, kb, pt8v)
                        if pending_pv is not None:
                            pending_pv()
                            pending_pv = None
                        if kb == 0:
                            pv_pair = (psV.tile([80, 512], f32, tag="pv",
                                                name=f"pvA_{t}_{c}"),
                                       psV.tile([80, 512], f32, tag="pv",
                                                name=f"pvB_{t}_{c}"))
                        if fp16diag:
                            def mk_pv(t=t, c=c, kb=kb, pt=pt,
                                      pv_pair=pv_pair):
                                emit_pv_diag(t, c, kb, pt, pv_pair)
                                if kb == nkb - 1:
                                    chunk_tail(t, c, pv_pair)
                            pending_pv = mk_pv
                        elif kb % 2 == 1:
                            jd0 = (kb - 1) - 4 * t
                            lo0 = 128 * jd0 if jd0 > 0 else 0
                            last = kb == nkb - 1
                            def mk_pv(t=t, c=c, kb0=kb - 1, pt8v=pt8v,
                                      pv_pair=pv_pair, lo0=lo0, last=last):
                                emit_pv_pair(t, c, kb0, pt8v, pv_pair,
                                             lo0=lo0, stop=last)
                                if last:
                                    chunk_tail(t, c, pv_pair)
                            pending_pv = mk_pv
                        for piece in fill_at.get(si, ()):
                            piece()
                    pending_pv()
                    pending_norm = [lambda t=t: recip_piece(t)]
                    pending_norm += norm_pieces(t)
                # tail: normalize + out-project the last tile
                for piece in pending_norm:
                    piece()
                for piece in outproj_pieces(NQT - 1):
                    piece()

    return nc


# ---------------------------------------------------------------------------
# host side
# ---------------------------------------------------------------------------
_PROG_CACHE = {}


def _get_program():
    if "nc" not in _PROG_CACHE:
        _install_axon_hooks()
        _install_drain_patch()
        _PROG_CACHE["nc"] = _build_program()
    return _PROG_CACHE["nc"]


def _prep_in_maps(inputs):
    x = np.asarray(inputs["x"], np.float32)
    pos = np.asarray(inputs["token_positions"]).astype(np.float32)
    WQ = np.asarray(inputs["W_Q"], np.float32)
    WK = np.asarray(inputs["W_K"], np.float32)
    WV = np.asarray(inputs["W_V"], np.float32)
    WO = np.asarray(inputs["W_O"], np.float32)

    # NeoX reorder of interleaved rope pairs, per head (rows of W_Q/W_K)
    perm = np.empty(D_MODEL, np.int64)
    for h in range(NUM_HEADS):
        b = h * HEAD_DIM
        perm[b:b + 32] = b + 2 * np.arange(32)
        perm[b + 32:b + 64] = b + 2 * np.arange(32) + 1
    # fp8 range rescale: W std is ~0.02, below the e4m3 normal range;
    # store 32x weights and fold 32*32 * HEAD_DIM**-0.5 into the exp scale
    f8 = ml_dtypes.float8_e4m3fn
    WQ32 = WQ[perm] * np.float32(32.0)
    WK32 = WK[perm] * np.float32(32.0)
    WV32 = WV * np.float32(32.0)
    WQp = WQ32.astype(f8)
    WKp = WK32.astype(f8)
    WVs = WV32.astype(f8)
    WQ16 = WQ32.astype(np.float16)
    WK16 = WK32.astype(np.float16)
    WV16 = WV32.astype(np.float16)
    x8 = x.astype(f8)
    x16 = x.astype(np.float16)

    # rope tables, mirroring the reference's float32 math
    j = np.arange(HEAD_DIM // 2, dtype=np.float32)
    inv_freq = np.power(np.float32(THETA),
                        (np.float32(-2.0) * j / np.float32(HEAD_DIM))
                        ).astype(np.float32)
    ang = pos[:, None] * inv_freq[None, :]          # (SEQ, 32) f32
    cos = np.cos(ang).astype(np.float32).T          # (32, SEQ)
    sin = np.sin(ang).astype(np.float32).T
    cos_t = np.ascontiguousarray(np.tile(cos, (4, 1))).astype(np.float16)
    sin_t = np.ascontiguousarray(
        np.concatenate([-sin, sin, -sin, sin], axis=0)).astype(np.float16)

    tri = (np.arange(128)[:, None] <= np.arange(128)[None, :])
    bigmask = tri.astype(np.float16)
    bigmask8 = tri.astype(ml_dtypes.float8_e4m3fn)
    ones64 = np.full((1, 64), 16.0 / 32.0, np.float32)
    halves65 = np.zeros((65, 128), np.float32)
    halves65[0, 0:64] = 16.0 / 32.0
    halves65[64, 64:128] = 16.0 / 32.0

    in_maps = []
    for core in range(N_CORES):
        b, g = core // 2, core % 2
        sl = slice(g * 512, (g + 1) * 512)
        in_maps.append({
            "xT": np.ascontiguousarray(x8[b].T),
            "wqT": np.ascontiguousarray(WQp[sl].T),
            "wkT": np.ascontiguousarray(WKp[sl].T),
            "wvT": np.ascontiguousarray(WVs[sl].T),
            "wq16T": np.ascontiguousarray(WQ16[sl].T),
            "wk16T": np.ascontiguousarray(WK16[sl].T),
            "wv16T": np.ascontiguousarray(WV16[sl].T),
            "x16T": np.ascontiguousarray(x16[b].T[:, 0:512]),
            "woT": np.ascontiguousarray(WO[:, sl].T.astype(np.float16)),
            "cos_t": cos_t,
            "sin_t": sin_t,
            "bigmask": bigmask,
            "bigmask8": bigmask8,
            "ones64": ones64,
            "halves65": halves65,
        })
    return in_maps


def kernel(**inputs):
    from concourse.bass_utils import run_bass_kernel_spmd

    nc = _get_program()
    if not _PROG_CACHE.get("waits_split"):
        _split_sync_waits(nc)
        _PROG_CACHE["waits_split"] = True
    in_maps = _prep_in_maps(inputs)
    trace = os.environ.get("BASS_KERNEL_TRACE") == "1"
    kw = {}
    if trace:
        kw = dict(trace=True, tmpdir=os.environ.get("BASS_KERNEL_TRACE_DIR"))
    res = run_bass_kernel_spmd(nc, in_maps, core_ids=list(range(N_CORES)), **kw)
    if trace:
        print(f"HW exec time: {res.exec_time_ns} ns "
              f"(mean {res.mean_exec_time_ns}, "
              f"max core {res.max_exec_time_core_id})")
        _PROG_CACHE["last_results"] = res

    out = np.empty((BATCH, SEQ, D_MODEL), np.float32)
    for b in range(BATCH):
        out[b] = res.results[2 * b]["out"] + res.results[2 * b + 1]["out"]
    return out
